# revision 23
# baseline (speedup 1.0000x reference)
"""Trainium2 Bass kernel for a dense transformer block (RoPE attention + SwiGLU).

Sharding (8 NeuronCores, Megatron-style):
  - QKV + attention: tensor-parallel over heads (2 heads/core, both batches).
  - Two half-AllToAlls (one per batch) reshard attention output from
    head-sharded to token-sharded; proj + SwiGLU MLP token-sharded.
v3 highlights vs v2:
  - Dual DMA rings: x/activations on the SP ring, weights on the ACT ring,
    so the QKV GEMM starts ~4us in instead of ~30us.
  - Softmax reciprocal on DVE (reciprocal_approx_fast) instead of ACT
    Reciprocal: the attention ScalarE stream is exp-only, no table reloads
    (v2 paid 17 ACT_TABLE_LOADs switching exp<->recip<->sigmoid).
  - proj(b0) + the b0-half SwiGLU GEMMs (N=256) are interleaved into the
    b1 attention passes so the PE stays dense while ScalarE chews exp
    (v2 oscillated the HAM clock gate 4/8<->13/16 every pass).
  - b0 silu deferred past attention (raw a@w1+b1 stored bf16); b1 MLP runs
    post-attention with a single fused Act.Silu.
  - a2a(1) hidden behind the remaining b0 MLP groups; w3 runs m-outer with
    resident weights, token-split, interleaved with the b1 MLP.
"""

import functools
import numpy as np
import ml_dtypes

B, T, C, H, D = 2, 2048, 1024, 16, 64
HID = 4 * C
NCORES = 8
HPC = H // NCORES          # heads per core
USE_DVE_RECIP = False      # softmax 1/z on DVE (else ACT Reciprocal)
WEIGHTS_ON_ACT_RING = True # weight DMAs on the ACT HWDGE ring


def _build_program(b, t):
    import concourse.bacc as bacc
    import concourse.mybir as mybir
    import concourse.tile as tile
    import concourse.masks as masks
    from contextlib import ExitStack

    fp32 = mybir.dt.float32
    bf16 = mybir.dt.bfloat16
    Act = mybir.ActivationFunctionType
    Alu = mybir.AluOpType

    tok = b * t                    # all tokens (b-major)
    tpc = tok // NCORES            # tokens per core for proj/MLP/out
    half = tpc // 2                # tokens per (core, batch)
    kt_tiles = t // 128            # 128-token key tiles per batch
    tchunks = tok // 128           # transpose chunks over all tokens
    qt_chunk = min(512, t)
    qt_chunks = t // qt_chunk      # P: passes per batch
    n_chunk = tok // 8             # QKV token chunks (8 PSUM banks)
    ck = C // 128                  # C chunks (8)
    mh_tiles = HID // 128          # hidden chunks (32)
    hg = 4                         # hidden chunks per weight-stream group
    n_groups = mh_tiles // hg      # 8 weight groups per batch-half
    gk = 2                         # key tiles per exp group
    g_cnt = kt_tiles // gk
    scale = float(D) ** -0.5

    nc = bacc.Bacc("TRN2", target_bir_lowering=False, debug=False,
                   num_devices=NCORES)
    wring = nc.scalar if WEIGHTS_ON_ACT_RING else nc.sync

    # ---- DRAM I/O ----
    xT_d = nc.dram_tensor("xT", [C, tok], bf16, kind="ExternalInput")
    wqkvT_d = nc.dram_tensor("wqkvT", [C, 3 * 128], bf16, kind="ExternalInput")
    bqkv_d = nc.dram_tensor("bqkv2d", [128, 3], fp32, kind="ExternalInput")
    cos_d = nc.dram_tensor("cosd", [128, tok], bf16, kind="ExternalInput")
    sin_d = nc.dram_tensor("sind", [128, tok], bf16, kind="ExternalInput")
    wprojT_d = nc.dram_tensor("wprojT", [C, C], bf16, kind="ExternalInput")
    bproj_d = nc.dram_tensor("bproj2d", [128, ck], fp32, kind="ExternalInput")
    w1T_d = nc.dram_tensor("w1T", [C, HID], bf16, kind="ExternalInput")
    w2T_d = nc.dram_tensor("w2T", [C, HID], bf16, kind="ExternalInput")
    w3T_d = nc.dram_tensor("w3T", [HID, C], bf16, kind="ExternalInput")
    b1_d = nc.dram_tensor("b1_2d", [128, mh_tiles], fp32, kind="ExternalInput")
    b2_d = nc.dram_tensor("b2_2d", [128, mh_tiles], fp32, kind="ExternalInput")
    b3_d = nc.dram_tensor("b3_2d", [128, ck], fp32, kind="ExternalInput")
    y_d = nc.dram_tensor("y_loc", [C, tpc], fp32, kind="ExternalOutput")

    with tile.TileContext(nc) as tc:
        es = ExitStack()
        # ---- constants / biases (live whole kernel; ACT-ring DMAs) ----
        consts = es.enter_context(tc.tile_pool(name="consts", bufs=1))
        ident = consts.tile([128, 128], bf16, name="ident")
        masks.make_identity(nc, ident[:])
        bqkv_sb = consts.tile([128, 3], fp32, name="bqkv_sb")
        wring.dma_start(out=bqkv_sb[:], in_=bqkv_d[:, :])
        bproj_sb = consts.tile([128, ck], fp32, name="bproj_sb")
        wring.dma_start(out=bproj_sb[:], in_=bproj_d[:, :])
        b1_sb = consts.tile([128, mh_tiles], fp32, name="b1_sb")
        wring.dma_start(out=b1_sb[:], in_=b1_d[:, :])
        b2_sb = consts.tile([128, mh_tiles], fp32, name="b2_sb")
        wring.dma_start(out=b2_sb[:], in_=b2_d[:, :])
        b3_sb = consts.tile([128, ck], fp32, name="b3_sb")
        wring.dma_start(out=b3_sb[:], in_=b3_d[:, :])

        # ---- DRAM bounce buffers for the two half-AllToAlls ----
        dram = es.enter_context(tc.tile_pool(name="dramp", bufs=1,
                                             space="DRAM"))
        a2a_in = [dram.tile([NCORES * 128, half], bf16, name=f"a2a_in{i}")
                  for i in range(2)]
        a2a_out = [dram.tile([NCORES * 128, half], bf16, name=f"a2a_out{i}")
                   for i in range(2)]

        # ---- attention-lifetime tensors ----
        glob = es.enter_context(tc.tile_pool(name="glob", bufs=1))
        kr = glob.tile([128, tok], bf16, name="kr")
        qr = glob.tile([128, tok], bf16, name="qr")
        v_bf = glob.tile([128, tok], bf16, name="v_bf")
        v_aug = glob.tile([128, tchunks * 130], bf16, name="v_aug")
        nc.vector.memset(v_aug[:], 1.0)
        out_all = glob.tile([128, tok], bf16, name="out_all")

        # ================= Phase A: QKV GEMM + RoPE =================
        # x tiles on the SP ring; weights + rope tables on the ACT ring so
        # the first matmul fires as soon as wq[0]+xt[0] land (~4us).
        ph_a = ExitStack()
        xt_pool = ph_a.enter_context(tc.tile_pool(name="xt", bufs=1))
        wq_pool = ph_a.enter_context(tc.tile_pool(name="wq", bufs=1))
        rope_tab = ph_a.enter_context(tc.tile_pool(name="ropetab", bufs=1))
        sw_pool = ph_a.enter_context(tc.tile_pool(name="swp", bufs=1))
        qk_pool = ph_a.enter_context(tc.tile_pool(name="qkp", bufs=1))
        ps_qkv = ph_a.enter_context(
            tc.tile_pool(name="ps_qkv", bufs=1, space="PSUM"))

        wq_sb = []
        for kc in range(ck):
            wq_kc = wq_pool.tile([128, 3 * 128], bf16, name=f"wqkv{kc}")
            wring.dma_start(out=wq_kc[:],
                                in_=wqkvT_d[128 * kc:128 * kc + 128, :])
            wq_sb.append(wq_kc)
        xt_sb = []
        for kc in range(ck):
            xt_kc = xt_pool.tile([128, tok], bf16, name=f"xt{kc}")
            nc.sync.dma_start(out=xt_kc[:], in_=xT_d[128 * kc:128 * kc + 128, :])
            xt_sb.append(xt_kc)
        cos_sb = rope_tab.tile([128, tok], bf16, name="cos_sb")
        wring.dma_start(out=cos_sb[:], in_=cos_d[:, :])
        sin_sb = rope_tab.tile([128, tok], bf16, name="sin_sb")
        wring.dma_start(out=sin_sb[:], in_=sin_d[:, :])

        k_bf = qk_pool.tile([128, tok], bf16, name="k_bf")
        q_bf = qk_pool.tile([128, tok], bf16, name="q_bf")
        ksw = sw_pool.tile([128, tok], bf16, name="ksw")
        qsw = sw_pool.tile([128, tok], bf16, name="qsw")

        ps_n = [ps_qkv.tile([128, n_chunk], fp32, name=f"psqkv{n}")
                for n in range(8)]
        dest = [k_bf, q_bf, v_bf]          # section order: k, q, v
        for mi in range(3):
            for kc in range(ck):
                wslice = wq_sb[kc][:, 128 * mi:128 * mi + 128]
                for n in range(8):
                    nc.tensor.matmul(
                        ps_n[n][:], wslice,
                        xt_sb[kc][:, n_chunk * n:n_chunk * (n + 1)],
                        start=(kc == 0), stop=(kc == ck - 1))
            for n in range(8):
                nc.scalar.activation(
                    dest[mi][:, n_chunk * n:n_chunk * (n + 1)], ps_n[n][:],
                    Act.Identity, bias=bqkv_sb[:, mi:mi + 1])
            if mi <= 1:
                # RoPE: half-swap via DMA, then r = u*cos + u_sw*sin_signed.
                u_bf, u_sw, u_r = dest[mi], (ksw, qsw)[mi], (kr, qr)[mi]
                for (so, do) in ((0, 32), (32, 0), (64, 96), (96, 64)):
                    nc.sync.dma_start(out=u_sw[do:do + 32, :],
                                      in_=u_bf[so:so + 32, :])
                nc.vector.tensor_mul(u_r[:], u_bf[:], cos_sb[:])
                nc.vector.tensor_mul(u_bf[:], u_sw[:], sin_sb[:])
                nc.vector.tensor_add(u_r[:], u_r[:], u_bf[:])
        ph_a.close()

        # ---- long-lived compute tensors (opened after phase A frees SBUF) ----
        es2 = ExitStack()
        wp_pool = es2.enter_context(tc.tile_pool(name="wpp", bufs=1))
        w1g_pool = es2.enter_context(tc.tile_pool(name="w1g", bufs=2))
        w2g_pool = es2.enter_context(tc.tile_pool(name="w2g", bufs=2))
        mlp_pool = es2.enter_context(tc.tile_pool(name="mlp", bufs=1))
        of_pool = es2.enter_context(tc.tile_pool(name="ofp", bufs=1))
        ht_pool = es2.enter_context(tc.tile_pool(name="htp", bufs=1))
        s_pool = es2.enter_context(tc.tile_pool(name="sp", bufs=2))
        # aT/hT split by batch-half to avoid false whole-tile deps
        aT = [mlp_pool.tile([128, ck * half], bf16, name=f"aT{i}")
              for i in range(2)]
        hT = [ht_pool.tile([128, mh_tiles * half], bf16, name=f"hT{i}")
              for i in range(2)]

        def raw_ap(mh):
            """Scratch slot for the deferred b0 pre-activation a@w1+b1.
            Reuses dead SBUF: v_bf after the transposes (mh 0-15), out_all
            after the a2a(1) staging reads (mh 16-31, bridge-emitted)."""
            if mh < 16:
                return v_bf[:, half * mh:half * (mh + 1)]
            return out_all[:, half * (mh - 16):half * (mh - 15)]

        # prefetch proj weights on the ACT ring during phase A tail
        wp_sb = []
        for kc in range(ck):
            wp_kc = wp_pool.tile([128, C], bf16, name=f"wp{kc}")
            wring.dma_start(out=wp_kc[:],
                                in_=wprojT_d[128 * kc:128 * kc + 128, :])
            wp_sb.append(wp_kc)

        # rolling w1/w2 group loads: w1 on ACT ring, w2 on SP ring.
        # stream index 0..2*n_groups-1: first b0's groups then b1's (reload).
        w1g_tiles = {}
        w2g_tiles = {}

        def load_mlp_group(s):
            g = s % n_groups
            w1g = []
            w2g = []
            for kc in range(ck):
                w1k = w1g_pool.tile([128, hg * 128], bf16,
                                    name=f"w1g{s}_{kc}", tag=f"w1g{kc}")
                wring.dma_start(
                    out=w1k[:],
                    in_=w1T_d[128 * kc:128 * kc + 128,
                              hg * 128 * g:hg * 128 * (g + 1)])
                w1g.append(w1k)
                w2k = w2g_pool.tile([128, hg * 128], bf16,
                                    name=f"w2g{s}_{kc}", tag=f"w2g{kc}")
                nc.sync.dma_start(
                    out=w2k[:],
                    in_=w2T_d[128 * kc:128 * kc + 128,
                              hg * 128 * g:hg * 128 * (g + 1)])
                w2g.append(w2k)
            w1g_tiles[s] = w1g
            w2g_tiles[s] = w2g

        load_mlp_group(0)
        load_mlp_group(1)
        next_load = [2]

        def prefetch_group():
            # Only ever called right after the consumers of the slot being
            # recycled were emitted, so the DMA's wait is always satisfiable
            # by already-issued instructions (no ring deadlock).
            if next_load[0] < 2 * n_groups:
                load_mlp_group(next_load[0])
                next_load[0] += 1

        # ---- v transpose into v_aug (PE + DVE copies) ----
        ph_tr = ExitStack()
        ps_tr = ph_tr.enter_context(
            tc.tile_pool(name="ps_tr", bufs=4, space="PSUM"))

        def emit_T(ci):
            pst = ps_tr.tile([128, 128], bf16, name=f"pst{ci}", tag="pst")
            nc.tensor.transpose(pst[:], v_bf[:, 128 * ci:128 * ci + 128],
                                ident[:])
            base = ci * 130
            nc.vector.tensor_copy(v_aug[:, base:base + 64], pst[:, 0:64])
            nc.vector.tensor_copy(v_aug[:, base + 65:base + 129],
                                  pst[:, 64:128])

        for ci in range(tchunks):
            emit_T(ci)
        ph_tr.close()

        # ================= Phase C: attention + interleaved proj/MLP ======
        ph_c = ExitStack()
        ps_g = ph_c.enter_context(tc.tile_pool(name="ps_g", bufs=2,
                                               space="PSUM"))
        ps_o = ph_c.enter_context(tc.tile_pool(name="ps_o", bufs=2,
                                               space="PSUM"))
        ps_m1 = ph_c.enter_context(tc.tile_pool(name="ps_m1", bufs=1,
                                                space="PSUM"))
        exp_pool = ph_c.enter_context(tc.tile_pool(name="expp", bufs=2))
        sm_pool = ph_c.enter_context(tc.tile_pool(name="smp", bufs=2))

        of_sb = [[None] * ck, [None] * ck]

        def emit_stage_cc(bi):
            for j in range(NCORES):
                c0 = bi * t + half * j
                nc.sync.dma_start(
                    out=a2a_in[bi][128 * j:128 * j + 128, :],
                    in_=out_all[:, c0:c0 + half])
            nc.gpsimd.collective_compute(
                "AllToAll", Alu.bypass,
                replica_groups=[list(range(NCORES))],
                ins=[a2a_in[bi][:]], outs=[a2a_out[bi][:]])

        def emit_of_load(bi):
            for kc in range(ck):
                of_kc = of_pool.tile([128, half], bf16, name=f"of{bi}_{kc}",
                                     tag=f"of{kc}")
                nc.sync.dma_start(out=of_kc[:],
                                  in_=a2a_out[bi][128 * kc:128 * kc + 128, :])
                of_sb[bi][kc] = of_kc

        def emit_proj_m(bi, m, pool):
            psa = pool.tile([128, half], fp32, name=f"psa{bi}_{m}",
                            tag=("ps1", "ps2")[m % 2])
            for kc in range(ck):
                nc.tensor.matmul(psa[:], wp_sb[kc][:, 128 * m:128 * m + 128],
                                 of_sb[bi][kc][:],
                                 start=(kc == 0), stop=(kc == ck - 1))
            nc.vector.tensor_scalar(
                out=aT[bi][:, half * m:half * (m + 1)],
                in0=psa[:], scalar1=bproj_sb[:, m:m + 1], scalar2=None,
                op0=Alu.add)

        def emit_mlp_mh(bi, mh, pool, inline_silu):
            """w1/w2 GEMMs for one hidden chunk over one batch-half.
            bi==0 (in-slot): store raw a@w1+b1 (silu deferred) and
            hT0 <- a@w2+b2.  bi==1: inline fused Silu."""
            s = (bi * n_groups) + mh // hg
            w1g = w1g_tiles[s]
            w2g = w2g_tiles[s]
            ml = mh % hg
            ps1 = pool.tile([128, half], fp32, name=f"ps1_{bi}_{mh}",
                            tag="ps1")
            ps2 = pool.tile([128, half], fp32, name=f"ps2_{bi}_{mh}",
                            tag="ps2")
            for kc in range(ck):
                nc.tensor.matmul(ps1[:],
                                 w1g[kc][:, 128 * ml:128 * ml + 128],
                                 aT[bi][:, half * kc:half * (kc + 1)],
                                 start=(kc == 0), stop=(kc == ck - 1))
            for kc in range(ck):
                nc.tensor.matmul(ps2[:],
                                 w2g[kc][:, 128 * ml:128 * ml + 128],
                                 aT[bi][:, half * kc:half * (kc + 1)],
                                 start=(kc == 0), stop=(kc == ck - 1))
            if inline_silu:
                g_sb = s_pool.tile([128, half], fp32, name=f"g{bi}_{mh}",
                                   tag="g")
                nc.scalar.activation(g_sb[:], ps1[:], Act.Sigmoid,
                                     bias=b1_sb[:, mh:mh + 1])
                s_sb = s_pool.tile([128, half], fp32, name=f"s{bi}_{mh}",
                                   tag="s")
                nc.vector.scalar_tensor_tensor(
                    s_sb[:], ps1[:], b1_sb[:, mh:mh + 1], g_sb[:],
                    op0=Alu.add, op1=Alu.mult)
                nc.vector.scalar_tensor_tensor(
                    hT[bi][:, half * mh:half * (mh + 1)], ps2[:],
                    b2_sb[:, mh:mh + 1], s_sb[:],
                    op0=Alu.add, op1=Alu.mult)
            else:
                nc.vector.tensor_scalar(
                    out=raw_ap(mh), in0=ps1[:],
                    scalar1=b1_sb[:, mh:mh + 1], scalar2=None, op0=Alu.add)
                nc.vector.tensor_scalar(
                    out=hT[bi][:, half * mh:half * (mh + 1)], in0=ps2[:],
                    scalar1=b2_sb[:, mh:mh + 1], scalar2=None, op0=Alu.add)
            if mh % hg == hg - 1:
                prefetch_group()

        # ---- slot plan: which extra PE work runs inside which b1 pass ----
        # pass P (first b1 pass): of_load(0) only (a2a(0) still in flight).
        # pass P+1: proj0 (8 m-chunks spread over the groups).
        # passes P+2..2P-1: one w1/w2 group (hg mh-chunks) each.
        P = qt_chunks
        mh_used = [0]          # next b0 mh chunk to emit in-slot
        proj0_done = [0]       # proj0 m-chunks emitted in-slot

        def slot_extra(pi, g):
            """Extra PE work injected after group g of pass pi."""
            if pi == P + 1:
                if proj0_done[0] < ck:
                    emit_proj_m(0, proj0_done[0], ps_m1)
                    proj0_done[0] += 1
            elif pi >= P + 2:
                # one mh per odd group: hg(=4) chunks over g_cnt(=8) groups
                if g % 2 == 1 and mh_used[0] < mh_tiles:
                    emit_mlp_mh(0, mh_used[0], ps_m1, False)
                    mh_used[0] += 1

        passes = [(bi, qc) for bi in range(b) for qc in range(qt_chunks)]
        for pi, (bi, qc) in enumerate(passes):
            q0 = bi * t + qc * qt_chunk
            horder = (0, 1) if pi % 2 == 0 else (1, 0)
            expTs = {}
            psos = {}
            for h in horder:
                expTs[h] = exp_pool.tile([128, kt_tiles * qt_chunk], bf16,
                                         name=f"expT{pi}_{h}", tag="expT")
                psos[h] = ps_o.tile([65, qt_chunk], fp32,
                                    name=f"pso{pi}_{h}", tag="pso")

            def emit_attnv(g):
                for h in horder:
                    for j in range(gk):
                        kt = gk * g + j
                        ci = bi * kt_tiles + kt
                        vblk = v_aug[:, ci * 130 + 65 * h:
                                     ci * 130 + 65 * h + 65]
                        nc.tensor.matmul(
                            psos[h][:], vblk,
                            expTs[h][:, qt_chunk * kt:qt_chunk * (kt + 1)],
                            start=(kt == 0), stop=(kt == kt_tiles - 1))

            if pi == P:
                emit_of_load(0)
                prefetch_group()
            for g in range(g_cnt):
                gps = {}
                for h in horder:
                    gps[h] = ps_g.tile([128, gk * qt_chunk], fp32,
                                       name=f"gps{pi}_{g}_{h}", tag="gps")
                # j-outer, h-inner: adjacent matmuls hit disjoint PE row
                # groups (h0 rows 0-63, h1 rows 64-127) and run concurrently
                for j in range(gk):
                    kt = gk * g + j
                    for h in horder:
                        nc.tensor.matmul(
                            gps[h][:, qt_chunk * j:qt_chunk * (j + 1)],
                            kr[64 * h:64 * h + 64,
                               bi * t + 128 * kt:bi * t + 128 * kt + 128],
                            qr[64 * h:64 * h + 64, q0:q0 + qt_chunk],
                            start=True, stop=True)
                for h in horder:
                    nc.scalar.activation(
                        expTs[h][:, gk * qt_chunk * g:gk * qt_chunk * (g + 1)],
                        gps[h][:], Act.Exp, scale=scale)
                slot_extra(pi, g)
                if g >= 1:
                    emit_attnv(g - 1)
            emit_attnv(g_cnt - 1)

            # softmax normalization: denominators live in pso row 64.
            # DVE approx-reciprocal (keeps ScalarE exp-only -> no ACT table
            # reloads), gpsimd broadcast to 64 partitions, DVE multiply.
            for h in horder:
                pso = psos[h]
                recip = sm_pool.tile([1, qt_chunk], fp32,
                                     name=f"rc{pi}_{h}", tag="rc")
                if USE_DVE_RECIP:
                    nc.vector.reciprocal_approx_fast(recip[0:1, :],
                                                     pso[64:65, :])
                else:
                    nc.scalar.add_instruction(mybir.InstActivation(
                        name=nc.get_next_instruction_name(),
                        func=Act.Reciprocal,
                        ins=[nc.scalar.lower_ap(pso[64:65, :]),
                             mybir.ImmediateValue(dtype=fp32, value=0.0),
                             mybir.ImmediateValue(dtype=fp32, value=1.0),
                             mybir.ImmediateValue(dtype=fp32, value=0.0)],
                        outs=[nc.scalar.lower_ap(recip[0:1, :])]))
                bcast = sm_pool.tile([64, qt_chunk], fp32,
                                     name=f"bc{pi}_{h}", tag="bc")
                nc.gpsimd.partition_broadcast(bcast[:], recip[0:1, :],
                                              channels=64)
                nc.vector.tensor_mul(
                    out_all[64 * h:64 * h + 64, q0:q0 + qt_chunk],
                    pso[0:64, :], bcast[:])

            if pi == P - 1:
                emit_stage_cc(0)

        emit_stage_cc(1)
        # attention PSUM freed for the dense MLP/w3 tail
        ph_c.close()
        es3 = ExitStack()
        w3_pool = es3.enter_context(tc.tile_pool(name="w3p", bufs=1))
        ph_c2 = ExitStack()
        # wider MLP double-buffering for the dense tail
        ps_m2 = ph_c2.enter_context(tc.tile_pool(name="ps_m2", bufs=2,
                                                 space="PSUM"))
        ps_3 = ph_c2.enter_context(tc.tile_pool(name="ps_3", bufs=2,
                                                space="PSUM"))
        out_pool = ph_c2.enter_context(tc.tile_pool(name="outp", bufs=2))
        w3_sb = [w3_pool.tile([128, C], bf16, name=f"w3k{kh}")
                 for kh in range(mh_tiles)]
        w3_cursor = [0]

        def emit_w3_loads(n):
            # w3 loads ride the ACT ring, doled out in chunks between the
            # rolling w1-group loads so neither starves the other.
            while n > 0 and w3_cursor[0] < mh_tiles:
                kh = w3_cursor[0]
                wring.dma_start(out=w3_sb[kh][:],
                                    in_=w3T_d[128 * kh:128 * kh + 128, :])
                w3_cursor[0] += 1
                n -= 1

        emit_of_load(1)
        # small-P fallback: proj0 chunks that never got an in-slot home
        for m in range(proj0_done[0], ck):
            emit_proj_m(0, m, ps_m2)

        # ---- bridge: remaining b0 MLP groups cover the a2a(1) latency ----
        while mh_used[0] < mh_tiles:
            emit_mlp_mh(0, mh_used[0], ps_m2, False)
            mh_used[0] += 1
            if mh_used[0] % hg == 0:
                emit_w3_loads(8)
        emit_w3_loads(mh_tiles)

        for m in range(ck):
            emit_proj_m(1, m, ps_m2)

        # deferred b0 silu: hT0 <- hT0 * raw * sigmoid(raw)  (ACT + DVE)
        for mh in range(mh_tiles):
            g_sb = s_pool.tile([128, half], fp32, name=f"gd{mh}", tag="g")
            nc.scalar.activation(g_sb[:], raw_ap(mh), Act.Sigmoid)
            s_sb = s_pool.tile([128, half], fp32, name=f"sd{mh}", tag="s")
            nc.vector.tensor_mul(s_sb[:], raw_ap(mh), g_sb[:])
            nc.vector.tensor_mul(hT[0][:, half * mh:half * (mh + 1)],
                                 hT[0][:, half * mh:half * (mh + 1)], s_sb[:])

        def emit_w3_half(bi):
            for m in range(ck):
                ps3 = ps_3.tile([128, half], fp32, name=f"ps3_{bi}_{m}",
                                tag="ps3")
                for kh in range(mh_tiles):
                    nc.tensor.matmul(ps3[:],
                                     w3_sb[kh][:, 128 * m:128 * m + 128],
                                     hT[bi][:, half * kh:half * (kh + 1)],
                                     start=(kh == 0), stop=(kh == mh_tiles - 1))
                yo = out_pool.tile([128, half], fp32, name=f"yo{bi}_{m}",
                                   tag="yo")
                nc.scalar.activation(yo[:], ps3[:], Act.Identity,
                                     bias=b3_sb[:, m:m + 1])
                nc.sync.dma_start(
                    out=y_d[128 * m:128 * m + 128, half * bi:half * (bi + 1)],
                    in_=yo[:])

        # b1 MLP (inline silu) interleaved with w3(b0)
        for mh in range(mh_tiles):
            emit_mlp_mh(1, mh, ps_m2, True)
            if mh == 2 * hg - 1:          # after 2 groups: silu-b0 done,
                emit_w3_half(0)           # w3 weights resident
        emit_w3_half(1)

        ph_c2.close()
        es3.close()
        es2.close()
        es.close()

    nc.compile()
    return nc


@functools.lru_cache(maxsize=2)
def _get_program(b, t):
    return _build_program(b, t)


def _prep_inputs(x, w_qkv, b_qkv, w_proj, b_proj, w1, b1, w2, b2, w3, b3,
                 cos, sin, b, t):
    """Build per-core in_maps (host-side sharding / transposes / casts)."""
    bf = ml_dtypes.bfloat16
    tok = b * t
    ck = C // 128
    mh_tiles = HID // 128

    xT = np.ascontiguousarray(x.reshape(tok, C).T).astype(bf)
    # RoPE tables tiled to [128, tok]: rows = 4x the 32 freq rows,
    # cols = b-major tokens.
    cosd = np.tile(cos.T, (4, b)).astype(bf)
    sind = np.tile(np.concatenate([-sin.T, sin.T], axis=0), (2, b)).astype(bf)
    wprojT = np.ascontiguousarray(w_proj.T).astype(bf)
    bproj2d = np.ascontiguousarray(b_proj.reshape(ck, 128).T).astype(np.float32)
    w1T = np.ascontiguousarray(w1.T).astype(bf)
    w2T = np.ascontiguousarray(w2.T).astype(bf)
    w3T = np.ascontiguousarray(w3.T).astype(bf)
    b1_2d = np.ascontiguousarray(b1.reshape(mh_tiles, 128).T).astype(np.float32)
    b2_2d = np.ascontiguousarray(b2.reshape(mh_tiles, 128).T).astype(np.float32)
    b3_2d = np.ascontiguousarray(b3.reshape(ck, 128).T).astype(np.float32)

    # even/odd RoPE permutation within each head's 64 dims
    perm = np.concatenate([np.arange(0, D, 2), np.arange(1, D, 2)])

    in_maps = []
    for c in range(NCORES):
        rows = []
        brows = []
        secperm = [(1, perm), (0, perm), (2, np.arange(D))]   # k, q, v
        for sec, p in secperm:
            for hh in range(HPC):
                h = HPC * c + hh
                idx = sec * H * D + h * D + p
                rows.append(w_qkv[idx, :])
                brows.append(b_qkv[idx])
        wql = np.concatenate(rows, axis=0)           # [384, C]
        bql = np.concatenate(brows, axis=0)          # [384]
        wqkvT = np.ascontiguousarray(wql.T).astype(bf)
        bqkv2d = np.ascontiguousarray(bql.reshape(3, 128).T).astype(np.float32)
        in_maps.append({
            "xT": xT, "wqkvT": wqkvT, "bqkv2d": bqkv2d,
            "cosd": cosd, "sind": sind,
            "wprojT": wprojT, "bproj2d": bproj2d,
            "w1T": w1T, "w2T": w2T, "w3T": w3T,
            "b1_2d": b1_2d, "b2_2d": b2_2d, "b3_2d": b3_2d,
        })
    return in_maps


def kernel(x, w_qkv, b_qkv, w_proj, b_proj, w1, b1, w2, b2, w3, b3, cos, sin,
           _trace=False):
    from concourse import bass_utils

    b, t, c = x.shape
    assert (b, t, c) == (B, T, C)
    args = [np.asarray(a, dtype=np.float32) for a in
            (x, w_qkv, b_qkv, w_proj, b_proj, w1, b1, w2, b2, w3, b3,
             cos, sin)]
    nc = _get_program(b, t)
    in_maps = _prep_inputs(*args, b, t)
    res = bass_utils.run_bass_kernel_spmd(
        nc, in_maps, core_ids=list(range(NCORES)), trace=_trace)
    tpc = (b * t) // NCORES
    half = tpc // 2
    y = np.empty((b * t, c), dtype=np.float32)
    for i in range(NCORES):
        yl = res.results[i]["y_loc"]
        for bi in range(b):
            y[bi * t + half * i: bi * t + half * (i + 1), :] = \
                yl[:, half * bi:half * (bi + 1)].T
    out = y.reshape(b, t, c)
    if _trace:
        return out, res
    return out
